# revision 43
# baseline (speedup 1.0000x reference)
"""Trainium2 Bass kernel for nn_ProtoCycleModel (retrieval_knn).

Problem: P=65536 prototypes, C=64 classes, D=256.
Per class c (rows c::64 of each table, n=1024):
    p2_inv = (p2_c - b) @ inv(W.T)          # y-side of direction "source"
    p1_fwd = p1_c @ W.T + b                 # y-side of direction "target"
    loss_src[c] = mean_i min_j ||p1_c[i] - p2_inv[j]||^2
    loss_tgt[c] = mean_i min_j ||p2_c[i] - p1_fwd[j]||^2
Output: (2, 64) fp32.

Sharding: class axis across 8 cores (8 classes/core).

Design (per core, per class, both directions dr in {0,1}):
  - Host passes class-major bf16 copies of both tables; the device loads
    them with XBAR DMA-transpose directly into d-major SBUF tiles
    xtb[t] = [128 d_lo, 2 d_chunk, 1024 i] bf16 (no PE transposes).
  - x~ = fp8(xtb) via gpsimd cast-DMA -> DoubleRow G stationary.
  - Transforms on PE: dir0 (through inv(W.T)) in bf16 (fp8 input noise
    would be amplified ~15x by the inverse); dir1 as fp8 DoubleRow
    (K=256 per instruction) reusing the fp8 p1 tile made for G.
  - yt[dr] = fp8(pstf + bias)  [Act]  -> DoubleRow G moving.
  - sq = Square(pstf*sqrt_c + bias*sqrt_c) [Act], ysrow ones-matmul (PE),
    ys_sb = copy to SBUF [Act].  ys = s_y*|y_j|^2 is computed from the
    UNquantized transform psum (critical for accuracy).
  - G tile per (dr, i-tile): one DoubleRow fp8 matmul per 512-bank
    (0.5 cycles/row), then a K=1 ones-matmul folds ys into the bank.
  - min_j(G + ys): DVE tensor_scalar(op1=min, accum_out) straight from
    PSUM for 11/16 i-tiles; for the other 5, Act copies psum->bf16 SBUF
    and the DVE min runs in 4x_2p mode (balances DVE vs Act, the two
    engines that can read PSUM; tensor_tensor_reduce and all gpsimd ALU
    ops are rejected by this toolchain).
  - Software pipelining: prep(c+1) interleaves with pairwise(c); DMA
    transposes prefetch two classes ahead; prep(0) is split so dr0
    min-units start while its dr1 half is still in flight.
  - Scalar |x_i|^2 term and all unscaling are applied on the host
    (loss = psf/(1024*s_y) + mean_i|x_i|^2).
"""

import numpy as np

P, C, D = 65536, 64, 256
N_CORES = 8
CPC = C // N_CORES          # classes per core = 8
NPC = P // C                # prototypes per class = 1024
IT = NPC // 128             # i-tiles per class = 8

_CACHE = {}


def _build_bass():
    import concourse.bass as bass
    from concourse import bacc
    import concourse.tile as tile
    from concourse import mybir

    FP32 = mybir.dt.float32
    FP32R = mybir.dt.float32r
    BF16 = mybir.dt.bfloat16
    FP8 = mybir.dt.float8e4
    AF = mybir.ActivationFunctionType
    ALU = mybir.AluOpType
    AX = mybir.AxisListType
    DR_MODE = mybir.MatmulPerfMode.DoubleRow

    nc = bacc.Bacc(None, target_bir_lowering=False)

    p1b_d = nc.dram_tensor("p1b", [CPC * NPC, D], BF16, kind="ExternalInput")
    p2b_d = nc.dram_tensor("p2b", [CPC * NPC, D], BF16, kind="ExternalInput")
    # mats[dr][dc][dcp] : [128,128] bf16 lhsT chunk of -2*s_y[dr]*T_dr
    mats_d = nc.dram_tensor("mats", [2, 2, 2, 128, 128], BF16,
                            kind="ExternalInput")
    # dir1 transform matrices in fp8 for DoubleRow (dc, dcp, d, d')
    mats1q_d = nc.dram_tensor("mats1q", [2, 2, 128, 128], FP8,
                              kind="ExternalInput")
    # constsr: [:,0:128] ones (ysrow lhsT / fold row), [:,128:129] ones col
    constsr_d = nc.dram_tensor("constsr", [128, 129], FP32R,
                               kind="ExternalInput")
    # constsf cols: 0-3 bias_dev[dr][dcp]; 4-7 bias_sq[dr][dcp]; 8-9 sqrt_c[dr]
    constsf_d = nc.dram_tensor("constsf", [128, 10], FP32, kind="ExternalInput")
    out_d = nc.dram_tensor("out", [1, 2 * CPC], FP32, kind="ExternalOutput")

    with tile.TileContext(nc) as tc:
        with (
            tc.tile_pool(name="const", bufs=1) as const,
            tc.tile_pool(name="xb", bufs=6) as xb_p,
            tc.tile_pool(name="xq", bufs=6) as xq_p,
            tc.tile_pool(name="yt", bufs=6) as yt_p,
            tc.tile_pool(name="sq", bufs=6) as sq_p,
            tc.tile_pool(name="ys", bufs=6) as ys_p,
            tc.tile_pool(name="gc", bufs=6) as gc_p,
            tc.tile_pool(name="psg", bufs=3, space="PSUM") as psg_p,
            tc.tile_pool(name="psm", bufs=1, space="PSUM") as psm_p,
        ):
            # const tiles; their DMAs are issued after the first class
            # loads so the table transposes get the early DMA sem slots.
            cr = const.tile([128, 129], FP32R)
            cf = const.tile([128, 10], FP32)
            mats = const.tile([128, 2, 2, 2, 128], BF16)
            mats1q = const.tile([128, 2, 2, 128], FP8)

            def emit_const_dmas():
                nc.scalar.dma_start(
                    mats[:], mats_d[:].rearrange("a b c p d -> p a b c d"))
                nc.scalar.dma_start(
                    mats1q[:], mats1q_d[:].rearrange("a b p d -> p a b d"))
                nc.scalar.dma_start(cf[:], constsf_d[:])
                nc.scalar.dma_start(cr[:], constsr_d[:])
            ones128 = cr[:, 0:128]
            ones_row = cr[0:1, 0:128]
            ones_col = cr[:, 128:129]

            # per-unit min columns: col = (ci*2 + dr)*8 + it.  All accums
            # come from DVE tensor_scalar ops (in-order engine).
            pmin = const.tile([128, 128], FP32, name="pmin")
            dumf = const.tile([128, 1], FP32, name="dumf")
            dumb = const.tile([128, 1], BF16, name="dumb")

            state = {}

            def emit_dmas(c):
                """DMA-transpose class-c rows of both tables into d-major
                (p2 first: dir0's transforms consume it first)."""
                halves = {}
                for t, src in ((1, p2b_d), (0, p1b_d)):
                    xtb = xb_p.tile([128, 2, NPC], BF16, tag=f"xb{t}",
                                    name=f"xtb{t}")
                    nc.sync.dma_start_transpose(
                        xtb[:], src[c * NPC:(c + 1) * NPC, :])
                    halves[t] = xtb
                state[("xtb", c)] = [halves[0], halves[1]]

            def prep_ops(c):
                """Generator of (engine_tag, thunk) prep ops for class c."""
                pp = psg_p if c == 0 else psm_p
                ptag = "g" if c == 0 else "psm"
                xtbs = state[("xtb", c)]
                xqs = [None, None]
                yts = [None, None]
                yss = [None, None]
                sqs = [[None, None], [None, None]]
                psys = [None, None]
                pstfs = [[None, None], [None, None]]
                state[("res", c)] = (xqs, yts, yss)

                def quant(t):
                    xq = xq_p.tile([128, 2, NPC], FP8, tag=f"xq{t}",
                                   name=f"xq{t}")
                    nc.gpsimd.dma_start(xq[:], xtbs[t][:])
                    xqs[t] = xq

                def tf(dr, dcp, dc, jh):
                    if dc == 0 and jh == 0:
                        pstfs[dr][dcp] = pp.tile(
                            [128, NPC], FP32, tag=ptag, name="pstf")
                    if dr == 1:
                        # dir1: fp8 DoubleRow transform, K=256 in one op,
                        # moving = the fp8 p1 tile already made for G.
                        if dc == 1:
                            return
                        nc.tensor.matmul(
                            pstfs[1][dcp][:, jh * 512:(jh + 1) * 512],
                            mats1q[:, :, dcp, :],
                            xqs[0][:, :, jh * 512:(jh + 1) * 512],
                            start=True, stop=True,
                            perf_mode=DR_MODE,
                        )
                        return
                    src = xtbs[1 - dr]  # dir0 transforms p2
                    nc.tensor.matmul(
                        pstfs[dr][dcp][:, jh * 512:(jh + 1) * 512],
                        mats[:, dr, dc, dcp, :],
                        src[:, dc, jh * 512:(jh + 1) * 512],
                        start=(dc == 0), stop=(dc == 1),
                    )

                def ytf(dr, dcp):
                    if dcp == 0:
                        yts[dr] = yt_p.tile([128, 2, NPC], FP8, tag=f"yt{dr}", name=f"yt{dr}")
                    nc.scalar.activation(
                        yts[dr][:, dcp, :], pstfs[dr][dcp][:], AF.Identity,
                        bias=cf[:, dr * 2 + dcp:dr * 2 + dcp + 1], scale=1.0)

                def sqf(dr, dcp):
                    sq_t = sq_p.tile([128, NPC], FP32R, tag="sq", name="sq_t")
                    nc.scalar.activation(
                        sq_t[:], pstfs[dr][dcp][:], AF.Square,
                        bias=cf[:, 4 + dr * 2 + dcp:4 + dr * 2 + dcp + 1],
                        scale=cf[:, 8 + dr:9 + dr])
                    sqs[dr][dcp] = sq_t

                def ysrow(dr, jh, dcp):
                    if jh == 0 and dcp == 0:
                        psys[dr] = pp.tile([128, NPC], FP32, tag=ptag, name="psy")
                    nc.tensor.matmul(
                        psys[dr][:, jh * 512:(jh + 1) * 512],
                        ones128,
                        sqs[dr][dcp][:, jh * 512:(jh + 1) * 512],
                        start=(dcp == 0), stop=(dcp == 1),
                    )

                def ysb(dr):
                    ys_t = ys_p.tile([128, NPC], FP32R, tag=f"ys{dr}", name=f"ys{dr}")
                    nc.scalar.copy(ys_t[:], psys[dr][:])
                    yss[dr] = ys_t

                # (engine, thunk) in dependency order; engine tags are only
                # informational for interleaving.
                ops = []
                ops.append(("act", lambda: quant(0)))
                ops.append(("act", lambda: quant(1)))
                for dr in range(2):
                    for dcp in range(2):
                        for jh in range(2):
                            for dc in range(2):
                                ops.append(("pe", (lambda a, b_, c_, d_:
                                            lambda: tf(a, b_, c_, d_))(
                                                dr, dcp, dc, jh)))
                        ops.append(("act", (lambda a, b_: lambda: ytf(a, b_))(
                            dr, dcp)))
                        ops.append(("act", (lambda a, b_: lambda: sqf(a, b_))(
                            dr, dcp)))
                    for jh in range(2):
                        for dcp in range(2):
                            ops.append(("pe", (lambda a, b_, c_:
                                        lambda: ysrow(a, b_, c_))(
                                            dr, jh, dcp)))
                    ops.append(("dve", (lambda a: lambda: ysb(a))(dr)))
                return ops

            def pairwise_units(c):
                """Generator of per-unit thunks for class c."""
                xqs, yts, yss = state.pop(("res", c))
                state.pop(("xtb", c))
                ci = c

                def unit(dr, it):
                    xside = xqs[dr]     # dir0 x = p1, dir1 x = p2
                    pg = psg_p.tile([128, NPC], FP32, tag="g", name="pg")
                    for jh in range(2):
                        # DoubleRow fp8 G (K=256 in one op), then fold ys
                        # into the same psum bank via a K=1 ones matmul.
                        nc.tensor.matmul(
                            pg[:, jh * 512:(jh + 1) * 512],
                            xside[:, :, it * 128:(it + 1) * 128],
                            yts[dr][:, :, jh * 512:(jh + 1) * 512],
                            start=True, stop=False,
                            perf_mode=DR_MODE,
                        )
                        nc.tensor.matmul(
                            pg[:, jh * 512:(jh + 1) * 512],
                            ones_row,
                            yss[dr][0:1, jh * 512:(jh + 1) * 512],
                            start=False, stop=True,
                        )
                    col = (ci * 2 + dr) * 8 + it
                    u = dr * 8 + it
                    if u % 3 != 2:
                        # direct: DVE min-reduce straight from PSUM
                        nc.vector.tensor_scalar(
                            out=dumf.broadcast_to((128, NPC)),
                            in0=pg[:], scalar1=0.0, scalar2=None,
                            op0=ALU.add, op1=ALU.min,
                            accum_out=pmin[:, col:col + 1])
                        return None
                    # offloaded: Act copies psum->bf16 SBUF; the DVE 4x min
                    # is deferred to the class end (keeps the DVE stream hot)
                    gc = gc_p.tile([128, NPC], BF16, tag="gc", name="gc")
                    nc.scalar.copy(gc[:], pg[:])
                    nc.vector.tensor_scalar(
                        out=dumb.broadcast_to((128, NPC)),
                        in0=gc[:], scalar1=0.0, scalar2=None,
                        op0=ALU.add, op1=ALU.min,
                        accum_out=pmin[:, col:col + 1])
                    return None

                return [(dr, it, (lambda a, b_: lambda: unit(a, b_))(dr, it))
                        for dr in (0, 1) for it in range(8)]

            # ---- software-pipelined main loop ----
            # D-path units write pminD cols with running-min per slot?  No:
            # each (dr, it) has its own (col, path) slot: it//2 in 0..3,
            # even it -> D, odd it -> P.  Each col written exactly twice?
            # it=0,2 -> slots 0,1 (D); it=4,6 -> slots 2,3 (D);
            # it=1,3 -> slots 0,1 (P); it=5,7 -> slots 2,3 (P).  Unique. OK.
            emit_dmas(0)
            emit_dmas(1)
            emit_const_dmas()
            # emit only the dr0 half of prep(0) up front; its dr1 half
            # rides in the first period's interleave so dr0 min-units can
            # start while dr1 prep is still in flight.
            prep_queue = prep_ops(0)
            n_dr0 = 2 + 8 + 2 + 2 + 4 + 1   # quant x2 + dir0 chain + ysb0
            for op in prep_queue[:n_dr0]:
                op[1]()
            carry = prep_queue[n_dr0:]
            for c in range(CPC):
                units = pairwise_units(c)
                if c + 2 < CPC:
                    emit_dmas(c + 2)
                if c + 1 < CPC:
                    prep_queue = carry + prep_ops(c + 1)
                else:
                    prep_queue = carry
                carry = []
                # interleave: after each unit, emit a slice of prep ops
                # (front-loaded into the first 12 unit slots); deferred
                # alpha-unit mins run at the end of the class.
                nu = len(units)
                np_ops = len(prep_queue)
                done = 0
                deferred = []
                for ui, (dr, it, thunk) in enumerate(units):
                    d = thunk()
                    if d is not None:
                        deferred.append(d)
                    want = min(np_ops, (ui + 1) * np_ops // 12)
                    while done < want:
                        prep_queue[done][1]()
                        done += 1
                for d in deferred:
                    d()

            # ---- finals ----
            # sum the 8 i-tiles per (class, dir), then cross-partition sum.
            red = const.tile([128, 16], FP32R, name="red")
            with nc.allow_low_precision(reason="fp32r is bit-identical fp32"):
                nc.vector.tensor_reduce(
                    out=red[:],
                    in_=pmin[:].rearrange("p (g k) -> p g k", k=8),
                    axis=AX.X, op=ALU.add)
            psf = psm_p.tile([1, 16], FP32, tag="psm", name="psf")
            nc.tensor.matmul(psf[:], ones_col, red[:], start=True, stop=True)
            outrow = const.tile([1, 16], FP32)
            nc.scalar.copy(outrow[:], psf[:])
            nc.sync.dma_start(out_d[:], outrow[:])

    nc.compile()
    return nc


def _get_nc():
    if "nc" not in _CACHE:
        _CACHE["nc"] = _build_bass()
    return _CACHE["nc"]


def kernel(protos1, protos2, W, b, num_classes):
    import ml_dtypes
    from concourse.bass_utils import run_bass_kernel_spmd

    nc_classes = int(num_classes)
    assert nc_classes == C and protos1.shape == (P, D)

    protos1 = np.ascontiguousarray(protos1, dtype=np.float32)
    protos2 = np.ascontiguousarray(protos2, dtype=np.float32)
    W = np.asarray(W, dtype=np.float32)
    b = np.asarray(b, dtype=np.float32)

    # host-side prep: inverse, scales, transform matrices
    V = np.linalg.inv(W.T.astype(np.float64)).astype(np.float32)  # (p2-b)@V
    B0 = (np.linalg.norm(protos2 - b, axis=1).max()
          * np.linalg.norm(V, axis=0).max())
    B1 = (np.linalg.norm(protos1, axis=1).max()
          * np.linalg.norm(W, axis=1).max() + np.abs(b).max())
    s_y = np.array([56.0 / B0, 56.0 / B1], np.float64)

    import ml_dtypes as _mld
    mats = np.zeros((2, 2, 2, 128, 128), _mld.bfloat16)
    for dr, T in ((0, V), (1, W.T.copy())):
        M = (-2.0 * s_y[dr]) * T.astype(np.float64)
        for dc in range(2):
            for dcp in range(2):
                mats[dr, dc, dcp] = M[dc * 128:(dc + 1) * 128,
                                      dcp * 128:(dcp + 1) * 128]

    mats1q = np.zeros((2, 2, 128, 128), np.float32)
    M1 = (-2.0 * s_y[1]) * W.T.astype(np.float64)
    for dc in range(2):
        for dcp in range(2):
            mats1q[dc, dcp] = M1[dc * 128:(dc + 1) * 128,
                                 dcp * 128:(dcp + 1) * 128]
    mats1q = mats1q.astype(ml_dtypes.float8_e4m3)

    bias_dev = np.zeros((2, 256), np.float64)
    bias_dev[0] = 2.0 * s_y[0] * (b.astype(np.float64) @ V.astype(np.float64))
    bias_dev[1] = -2.0 * s_y[1] * b
    sqrt_c = np.sqrt(1.0 / (4.0 * s_y))          # per dir

    constsf = np.zeros((128, 10), np.float32)
    for dr in range(2):
        for dcp in range(2):
            col = bias_dev[dr, dcp * 128:(dcp + 1) * 128]
            constsf[:, dr * 2 + dcp] = col
            constsf[:, 4 + dr * 2 + dcp] = col * sqrt_c[dr]
        constsf[:, 8 + dr] = sqrt_c[dr]
    constsr = np.concatenate(
        [np.ones((128, 128), np.float32), np.ones((128, 1), np.float32)],
        axis=1)

    # class-major reordering: (P, D) -> (C, NPC, D), bf16 copies
    p1c = np.ascontiguousarray(protos1.reshape(NPC, C, D).transpose(1, 0, 2))
    p2c = np.ascontiguousarray(protos2.reshape(NPC, C, D).transpose(1, 0, 2))
    p1bf = p1c.astype(ml_dtypes.bfloat16)
    p2bf = p2c.astype(ml_dtypes.bfloat16)

    # host-side |x|^2 means per (dir, class)
    xs0 = (p1c.astype(np.float64) ** 2).sum(axis=2).mean(axis=1)  # (C,)
    xs1 = (p2c.astype(np.float64) ** 2).sum(axis=2).mean(axis=1)

    in_maps = []
    for core in range(N_CORES):
        sl = slice(core * CPC, (core + 1) * CPC)
        in_maps.append({
            "p1b": np.ascontiguousarray(p1bf[sl].reshape(CPC * NPC, D)),
            "p2b": np.ascontiguousarray(p2bf[sl].reshape(CPC * NPC, D)),
            "mats": mats,
            "mats1q": mats1q,
            "constsr": constsr,
            "constsf": constsf,
        })

    nc = _get_nc()
    res = run_bass_kernel_spmd(nc, in_maps, core_ids=list(range(N_CORES)))
    _CACHE["last_result"] = res

    out = np.zeros((2, C), dtype=np.float64)
    for core in range(N_CORES):
        row = res.results[core]["out"].reshape(CPC, 2).astype(np.float64)
        for dr in range(2):
            out[dr, core * CPC:(core + 1) * CPC] = row[:, dr] / (NPC * s_y[dr])
    out[0] += xs0
    out[1] += xs1
    return out.astype(np.float32)


# revision 44
# speedup vs baseline: 1.0014x; 1.0014x over previous
"""Trainium2 Bass kernel for nn_ProtoCycleModel (retrieval_knn).

Problem: P=65536 prototypes, C=64 classes, D=256.
Per class c (rows c::64 of each table, n=1024):
    p2_inv = (p2_c - b) @ inv(W.T)          # y-side of direction "source"
    p1_fwd = p1_c @ W.T + b                 # y-side of direction "target"
    loss_src[c] = mean_i min_j ||p1_c[i] - p2_inv[j]||^2
    loss_tgt[c] = mean_i min_j ||p2_c[i] - p1_fwd[j]||^2
Output: (2, 64) fp32.

Sharding: class axis across 8 cores (8 classes/core).

Design (per core, per class, both directions dr in {0,1}):
  - Host passes class-major bf16 copies of both tables; the device loads
    them with XBAR DMA-transpose directly into d-major SBUF tiles
    xtb[t] = [128 d_lo, 2 d_chunk, 1024 i] bf16 (no PE transposes).
  - x~ = fp8(xtb) via gpsimd cast-DMA -> DoubleRow G stationary.
  - Transforms on PE: dir0 (through inv(W.T)) in bf16 (fp8 input noise
    would be amplified ~15x by the inverse); dir1 as fp8 DoubleRow
    (K=256 per instruction) reusing the fp8 p1 tile made for G.
  - yt[dr] = fp8(pstf + bias)  [Act]  -> DoubleRow G moving.
  - sq = Square(pstf*sqrt_c + bias*sqrt_c) [Act], ysrow ones-matmul (PE),
    ys_sb = copy to SBUF [Act].  ys = s_y*|y_j|^2 is computed from the
    UNquantized transform psum (critical for accuracy).
  - G tile per (dr, i-tile): one DoubleRow fp8 matmul per 512-bank
    (0.5 cycles/row), then a K=1 ones-matmul folds ys into the bank.
  - min_j(G + ys): DVE tensor_scalar(op1=min, accum_out) straight from
    PSUM for 11/16 i-tiles; for the other 5, Act copies psum->bf16 SBUF
    and the DVE min runs in 4x_2p mode (balances DVE vs Act, the two
    engines that can read PSUM; tensor_tensor_reduce and all gpsimd ALU
    ops are rejected by this toolchain).
  - Software pipelining: prep(c+1) interleaves with pairwise(c); DMA
    transposes prefetch two classes ahead; prep(0) is split so dr0
    min-units start while its dr1 half is still in flight.
  - Scalar |x_i|^2 term and all unscaling are applied on the host
    (loss = psf/(1024*s_y) + mean_i|x_i|^2).
"""

import numpy as np

P, C, D = 65536, 64, 256
N_CORES = 8
CPC = C // N_CORES          # classes per core = 8
NPC = P // C                # prototypes per class = 1024
IT = NPC // 128             # i-tiles per class = 8

_CACHE = {}


def _build_bass():
    import concourse.bass as bass
    from concourse import bacc
    import concourse.tile as tile
    from concourse import mybir

    FP32 = mybir.dt.float32
    FP32R = mybir.dt.float32r
    BF16 = mybir.dt.bfloat16
    FP8 = mybir.dt.float8e4
    AF = mybir.ActivationFunctionType
    ALU = mybir.AluOpType
    AX = mybir.AxisListType
    DR_MODE = mybir.MatmulPerfMode.DoubleRow

    nc = bacc.Bacc(None, target_bir_lowering=False)

    p1b_d = nc.dram_tensor("p1b", [CPC * NPC, D], BF16, kind="ExternalInput")
    p2b_d = nc.dram_tensor("p2b", [CPC * NPC, D], BF16, kind="ExternalInput")
    # mats[dr][dc][dcp] : [128,128] bf16 lhsT chunk of -2*s_y[dr]*T_dr
    mats_d = nc.dram_tensor("mats", [2, 2, 2, 128, 128], BF16,
                            kind="ExternalInput")
    # dir1 transform matrices in fp8 for DoubleRow (dc, dcp, d, d')
    mats1q_d = nc.dram_tensor("mats1q", [2, 2, 128, 128], FP8,
                              kind="ExternalInput")
    # constsr: [:,0:128] ones (ysrow lhsT / fold row), [:,128:129] ones col
    constsr_d = nc.dram_tensor("constsr", [128, 129], FP32R,
                               kind="ExternalInput")
    # constsf cols: 0-3 bias_dev[dr][dcp]; 4-7 bias_sq[dr][dcp]; 8-9 sqrt_c[dr]
    constsf_d = nc.dram_tensor("constsf", [128, 10], FP32, kind="ExternalInput")
    out_d = nc.dram_tensor("out", [1, 2 * CPC], FP32, kind="ExternalOutput")

    with tile.TileContext(nc) as tc:
        with (
            tc.tile_pool(name="const", bufs=1) as const,
            tc.tile_pool(name="xb", bufs=6) as xb_p,
            tc.tile_pool(name="xq", bufs=6) as xq_p,
            tc.tile_pool(name="yt", bufs=6) as yt_p,
            tc.tile_pool(name="sq", bufs=6) as sq_p,
            tc.tile_pool(name="ys", bufs=6) as ys_p,
            tc.tile_pool(name="gc", bufs=6) as gc_p,
            tc.tile_pool(name="psg", bufs=3, space="PSUM") as psg_p,
            tc.tile_pool(name="psm", bufs=1, space="PSUM") as psm_p,
        ):
            # const tiles; their DMAs are issued after the first class
            # loads so the table transposes get the early DMA sem slots.
            cr = const.tile([128, 129], FP32R)
            cf = const.tile([128, 10], FP32)
            mats = const.tile([128, 2, 2, 2, 128], BF16)
            mats1q = const.tile([128, 2, 2, 128], FP8)

            def emit_const_dmas():
                nc.scalar.dma_start(
                    mats[:], mats_d[:].rearrange("a b c p d -> p a b c d"))
                nc.scalar.dma_start(
                    mats1q[:], mats1q_d[:].rearrange("a b p d -> p a b d"))
                nc.scalar.dma_start(cf[:], constsf_d[:])
                nc.scalar.dma_start(cr[:], constsr_d[:])
            ones128 = cr[:, 0:128]
            ones_row = cr[0:1, 0:128]
            ones_col = cr[:, 128:129]

            # per-unit min columns: col = (ci*2 + dr)*8 + it.  All accums
            # come from DVE tensor_scalar ops (in-order engine).
            pmin = const.tile([128, 128], FP32, name="pmin")
            dumf = const.tile([128, 1], FP32, name="dumf")
            dumb = const.tile([128, 1], BF16, name="dumb")

            state = {}

            def emit_dmas(c):
                """DMA-transpose class-c rows of both tables into d-major
                (p2 first: dir0's transforms consume it first)."""
                halves = {}
                for t, src in ((1, p2b_d), (0, p1b_d)):
                    xtb = xb_p.tile([128, 2, NPC], BF16, tag=f"xb{t}",
                                    name=f"xtb{t}")
                    nc.sync.dma_start_transpose(
                        xtb[:], src[c * NPC:(c + 1) * NPC, :])
                    halves[t] = xtb
                state[("xtb", c)] = [halves[0], halves[1]]

            def prep_ops(c):
                """Generator of (engine_tag, thunk) prep ops for class c."""
                pp = psg_p if c == 0 else psm_p
                ptag = "g" if c == 0 else "psm"
                xtbs = state[("xtb", c)]
                xqs = [None, None]
                yts = [None, None]
                yss = [None, None]
                sqs = [[None, None], [None, None]]
                psys = [None, None]
                pstfs = [[None, None], [None, None]]
                state[("res", c)] = (xqs, yts, yss)

                def quant(t):
                    xq = xq_p.tile([128, 2, NPC], FP8, tag=f"xq{t}",
                                   name=f"xq{t}")
                    nc.gpsimd.dma_start(xq[:], xtbs[t][:])
                    xqs[t] = xq

                def tf(dr, dcp, dc, jh):
                    if dc == 0 and jh == 0:
                        pstfs[dr][dcp] = pp.tile(
                            [128, NPC], FP32, tag=ptag, name="pstf")
                    if dr == 1:
                        # dir1: fp8 DoubleRow transform, K=256 in one op,
                        # moving = the fp8 p1 tile already made for G.
                        if dc == 1:
                            return
                        nc.tensor.matmul(
                            pstfs[1][dcp][:, jh * 512:(jh + 1) * 512],
                            mats1q[:, :, dcp, :],
                            xqs[0][:, :, jh * 512:(jh + 1) * 512],
                            start=True, stop=True,
                            perf_mode=DR_MODE,
                        )
                        return
                    src = xtbs[1 - dr]  # dir0 transforms p2
                    nc.tensor.matmul(
                        pstfs[dr][dcp][:, jh * 512:(jh + 1) * 512],
                        mats[:, dr, dc, dcp, :],
                        src[:, dc, jh * 512:(jh + 1) * 512],
                        start=(dc == 0), stop=(dc == 1),
                    )

                def ytf(dr, dcp):
                    if dcp == 0:
                        yts[dr] = yt_p.tile([128, 2, NPC], FP8, tag=f"yt{dr}", name=f"yt{dr}")
                    nc.scalar.activation(
                        yts[dr][:, dcp, :], pstfs[dr][dcp][:], AF.Identity,
                        bias=cf[:, dr * 2 + dcp:dr * 2 + dcp + 1], scale=1.0)

                def sqf(dr, dcp):
                    sq_t = sq_p.tile([128, NPC], FP32R, tag="sq", name="sq_t")
                    nc.scalar.activation(
                        sq_t[:], pstfs[dr][dcp][:], AF.Square,
                        bias=cf[:, 4 + dr * 2 + dcp:4 + dr * 2 + dcp + 1],
                        scale=cf[:, 8 + dr:9 + dr])
                    sqs[dr][dcp] = sq_t

                def ysrow(dr, jh, dcp):
                    if jh == 0 and dcp == 0:
                        psys[dr] = pp.tile([128, NPC], FP32, tag=ptag, name="psy")
                    nc.tensor.matmul(
                        psys[dr][:, jh * 512:(jh + 1) * 512],
                        ones128,
                        sqs[dr][dcp][:, jh * 512:(jh + 1) * 512],
                        start=(dcp == 0), stop=(dcp == 1),
                    )

                def ysb(dr):
                    ys_t = ys_p.tile([128, NPC], FP32R, tag=f"ys{dr}", name=f"ys{dr}")
                    nc.scalar.copy(ys_t[:], psys[dr][:])
                    yss[dr] = ys_t

                # (engine, thunk) in dependency order; engine tags are only
                # informational for interleaving.
                ops = []
                ops.append(("act", lambda: quant(0)))
                ops.append(("act", lambda: quant(1)))
                for dr in range(2):
                    for dcp in range(2):
                        for jh in range(2):
                            for dc in range(2):
                                ops.append(("pe", (lambda a, b_, c_, d_:
                                            lambda: tf(a, b_, c_, d_))(
                                                dr, dcp, dc, jh)))
                        ops.append(("act", (lambda a, b_: lambda: sqf(a, b_))(
                            dr, dcp)))
                        ops.append(("act", (lambda a, b_: lambda: ytf(a, b_))(
                            dr, dcp)))
                    for jh in range(2):
                        for dcp in range(2):
                            ops.append(("pe", (lambda a, b_, c_:
                                        lambda: ysrow(a, b_, c_))(
                                            dr, jh, dcp)))
                    ops.append(("dve", (lambda a: lambda: ysb(a))(dr)))
                return ops

            def pairwise_units(c):
                """Generator of per-unit thunks for class c."""
                xqs, yts, yss = state.pop(("res", c))
                state.pop(("xtb", c))
                ci = c

                def unit(dr, it):
                    xside = xqs[dr]     # dir0 x = p1, dir1 x = p2
                    pg = psg_p.tile([128, NPC], FP32, tag="g", name="pg")
                    for jh in range(2):
                        # DoubleRow fp8 G (K=256 in one op), then fold ys
                        # into the same psum bank via a K=1 ones matmul.
                        nc.tensor.matmul(
                            pg[:, jh * 512:(jh + 1) * 512],
                            xside[:, :, it * 128:(it + 1) * 128],
                            yts[dr][:, :, jh * 512:(jh + 1) * 512],
                            start=True, stop=False,
                            perf_mode=DR_MODE,
                        )
                        nc.tensor.matmul(
                            pg[:, jh * 512:(jh + 1) * 512],
                            ones_row,
                            yss[dr][0:1, jh * 512:(jh + 1) * 512],
                            start=False, stop=True,
                        )
                    col = (ci * 2 + dr) * 8 + it
                    u = dr * 8 + it
                    if u % 3 != 2:
                        # direct: DVE min-reduce straight from PSUM
                        nc.vector.tensor_scalar(
                            out=dumf.broadcast_to((128, NPC)),
                            in0=pg[:], scalar1=0.0, scalar2=None,
                            op0=ALU.add, op1=ALU.min,
                            accum_out=pmin[:, col:col + 1])
                        return None
                    # offloaded: Act copies psum->bf16 SBUF; the DVE 4x min
                    # is deferred to the class end (keeps the DVE stream hot)
                    gc = gc_p.tile([128, NPC], BF16, tag="gc", name="gc")
                    nc.scalar.copy(gc[:], pg[:])
                    nc.vector.tensor_scalar(
                        out=dumb.broadcast_to((128, NPC)),
                        in0=gc[:], scalar1=0.0, scalar2=None,
                        op0=ALU.add, op1=ALU.min,
                        accum_out=pmin[:, col:col + 1])
                    return None

                return [(dr, it, (lambda a, b_: lambda: unit(a, b_))(dr, it))
                        for dr in (0, 1) for it in range(8)]

            # ---- software-pipelined main loop ----
            # D-path units write pminD cols with running-min per slot?  No:
            # each (dr, it) has its own (col, path) slot: it//2 in 0..3,
            # even it -> D, odd it -> P.  Each col written exactly twice?
            # it=0,2 -> slots 0,1 (D); it=4,6 -> slots 2,3 (D);
            # it=1,3 -> slots 0,1 (P); it=5,7 -> slots 2,3 (P).  Unique. OK.
            emit_dmas(0)
            emit_dmas(1)
            emit_const_dmas()
            # emit only the dr0 half of prep(0) up front; its dr1 half
            # rides in the first period's interleave so dr0 min-units can
            # start while dr1 prep is still in flight.
            prep_queue = prep_ops(0)
            n_dr0 = 2 + 8 + 2 + 2 + 4 + 1   # quant x2 + dir0 chain + ysb0
            for op in prep_queue[:n_dr0]:
                op[1]()
            carry = prep_queue[n_dr0:]
            for c in range(CPC):
                units = pairwise_units(c)
                if c + 2 < CPC:
                    emit_dmas(c + 2)
                if c + 1 < CPC:
                    prep_queue = carry + prep_ops(c + 1)
                else:
                    prep_queue = carry
                carry = []
                # interleave: after each unit, emit a slice of prep ops
                # (front-loaded into the first 12 unit slots); deferred
                # alpha-unit mins run at the end of the class.
                nu = len(units)
                np_ops = len(prep_queue)
                done = 0
                deferred = []
                for ui, (dr, it, thunk) in enumerate(units):
                    d = thunk()
                    if d is not None:
                        deferred.append(d)
                    want = min(np_ops, (ui + 1) * np_ops // 12)
                    while done < want:
                        prep_queue[done][1]()
                        done += 1
                for d in deferred:
                    d()

            # ---- finals ----
            # sum the 8 i-tiles per (class, dir), then cross-partition sum.
            red = const.tile([128, 16], FP32R, name="red")
            with nc.allow_low_precision(reason="fp32r is bit-identical fp32"):
                nc.vector.tensor_reduce(
                    out=red[:],
                    in_=pmin[:].rearrange("p (g k) -> p g k", k=8),
                    axis=AX.X, op=ALU.add)
            psf = psm_p.tile([1, 16], FP32, tag="psm", name="psf")
            nc.tensor.matmul(psf[:], ones_col, red[:], start=True, stop=True)
            outrow = const.tile([1, 16], FP32)
            nc.scalar.copy(outrow[:], psf[:])
            nc.sync.dma_start(out_d[:], outrow[:])

    nc.compile()
    return nc


def _get_nc():
    if "nc" not in _CACHE:
        _CACHE["nc"] = _build_bass()
    return _CACHE["nc"]


def kernel(protos1, protos2, W, b, num_classes):
    import ml_dtypes
    from concourse.bass_utils import run_bass_kernel_spmd

    nc_classes = int(num_classes)
    assert nc_classes == C and protos1.shape == (P, D)

    protos1 = np.ascontiguousarray(protos1, dtype=np.float32)
    protos2 = np.ascontiguousarray(protos2, dtype=np.float32)
    W = np.asarray(W, dtype=np.float32)
    b = np.asarray(b, dtype=np.float32)

    # host-side prep: inverse, scales, transform matrices
    V = np.linalg.inv(W.T.astype(np.float64)).astype(np.float32)  # (p2-b)@V
    B0 = (np.linalg.norm(protos2 - b, axis=1).max()
          * np.linalg.norm(V, axis=0).max())
    B1 = (np.linalg.norm(protos1, axis=1).max()
          * np.linalg.norm(W, axis=1).max() + np.abs(b).max())
    s_y = np.array([56.0 / B0, 56.0 / B1], np.float64)

    import ml_dtypes as _mld
    mats = np.zeros((2, 2, 2, 128, 128), _mld.bfloat16)
    for dr, T in ((0, V), (1, W.T.copy())):
        M = (-2.0 * s_y[dr]) * T.astype(np.float64)
        for dc in range(2):
            for dcp in range(2):
                mats[dr, dc, dcp] = M[dc * 128:(dc + 1) * 128,
                                      dcp * 128:(dcp + 1) * 128]

    mats1q = np.zeros((2, 2, 128, 128), np.float32)
    M1 = (-2.0 * s_y[1]) * W.T.astype(np.float64)
    for dc in range(2):
        for dcp in range(2):
            mats1q[dc, dcp] = M1[dc * 128:(dc + 1) * 128,
                                 dcp * 128:(dcp + 1) * 128]
    mats1q = mats1q.astype(ml_dtypes.float8_e4m3)

    bias_dev = np.zeros((2, 256), np.float64)
    bias_dev[0] = 2.0 * s_y[0] * (b.astype(np.float64) @ V.astype(np.float64))
    bias_dev[1] = -2.0 * s_y[1] * b
    sqrt_c = np.sqrt(1.0 / (4.0 * s_y))          # per dir

    constsf = np.zeros((128, 10), np.float32)
    for dr in range(2):
        for dcp in range(2):
            col = bias_dev[dr, dcp * 128:(dcp + 1) * 128]
            constsf[:, dr * 2 + dcp] = col
            constsf[:, 4 + dr * 2 + dcp] = col * sqrt_c[dr]
        constsf[:, 8 + dr] = sqrt_c[dr]
    constsr = np.concatenate(
        [np.ones((128, 128), np.float32), np.ones((128, 1), np.float32)],
        axis=1)

    # class-major reordering: (P, D) -> (C, NPC, D), bf16 copies
    p1c = np.ascontiguousarray(protos1.reshape(NPC, C, D).transpose(1, 0, 2))
    p2c = np.ascontiguousarray(protos2.reshape(NPC, C, D).transpose(1, 0, 2))
    p1bf = p1c.astype(ml_dtypes.bfloat16)
    p2bf = p2c.astype(ml_dtypes.bfloat16)

    # host-side |x|^2 means per (dir, class)
    xs0 = (p1c.astype(np.float64) ** 2).sum(axis=2).mean(axis=1)  # (C,)
    xs1 = (p2c.astype(np.float64) ** 2).sum(axis=2).mean(axis=1)

    in_maps = []
    for core in range(N_CORES):
        sl = slice(core * CPC, (core + 1) * CPC)
        in_maps.append({
            "p1b": np.ascontiguousarray(p1bf[sl].reshape(CPC * NPC, D)),
            "p2b": np.ascontiguousarray(p2bf[sl].reshape(CPC * NPC, D)),
            "mats": mats,
            "mats1q": mats1q,
            "constsr": constsr,
            "constsf": constsf,
        })

    nc = _get_nc()
    res = run_bass_kernel_spmd(nc, in_maps, core_ids=list(range(N_CORES)))
    _CACHE["last_result"] = res

    out = np.zeros((2, C), dtype=np.float64)
    for core in range(N_CORES):
        row = res.results[core]["out"].reshape(CPC, 2).astype(np.float64)
        for dr in range(2):
            out[dr, core * CPC:(core + 1) * CPC] = row[:, dr] / (NPC * s_y[dr])
    out[0] += xs0
    out[1] += xs1
    return out.astype(np.float32)
